# revision 11
# baseline (speedup 1.0000x reference)
"""Trainium2 Bass kernel for CustomGINE message passing (8 NeuronCores).

Strategy:
  - Nodes are sharded by destination across the 8 cores (12500 nodes each);
    each core receives exactly the edges whose dst falls in its node range,
    so the per-node aggregation is fully local (no big all-reduce).
  - Within a core, edge slots are laid out by (src supergroup q of 32768
    nodes, dst block b of 128 nodes) so that:
      * x[src] rows (256B each) are fetched with big `dma_gather` calls
        using int16 indices relative to the supergroup base, and
      * the segment-sum becomes, per 128-edge tile, one matmul
        msg^T @ S where S[e, j] = (dst_e == block_start + j) is built
        on-device in bf16 with a tensor-tensor is_equal.
  - edge_attr embedding rows are added via a K=4 one-hot matmul into the
    same PSUM accumulation (hi+lo bf16 split of the embedding table keeps
    full precision).
  - The MLP tail runs in feature-major (transposed) layout so BatchNorm
    statistics are free-dim reductions; the cross-core BN mean/var
    reduction is a tiny [64, 2] AllReduce collective.

Only index/layout work happens on the host (sorting, padding, one-hot
encodings, dtype of index tensors); all floating-point math runs on device.
"""

import os
import sys

for _p in ("/opt/trn_rl_repo", "/root/.axon_site/_ro/trn_rl_repo"):
    if os.path.isdir(_p) and _p not in sys.path:
        sys.path.insert(0, _p)

import numpy as np
import ml_dtypes

BF16 = ml_dtypes.bfloat16

N_NODES = 100000
IN_DIM = 64
NCORES = 8
NPC = N_NODES // NCORES          # nodes per core
P = 128                          # partitions
NBLK = (NPC + P - 1) // P        # dst blocks of 128 nodes per core (98)
NLOC = NBLK * P                  # padded local node count (12544)
Q = 4                            # src supergroups
QS = 25000                       # supergroup stride (balanced; < 32768)
QW = 32768                       # gather window rows per supergroup
XROWS = 3 * QS + QW              # padded gather table rows (107768)
BN_EPS = 1e-5

_PROGRAM_CACHE = {}


def _host_prepare(x, edge_index, edge_attr):
    """Shard + lay out edges; returns per-core index/metadata arrays."""
    src = np.asarray(edge_index[0], dtype=np.int64)
    dst = np.asarray(edge_index[1], dtype=np.int64)
    attr = np.asarray(edge_attr, dtype=np.int64)
    E = src.shape[0]

    core = dst // NPC
    q = src // QS
    dloc = dst - core * NPC
    b = dloc >> 7

    # order edges by (core, q, b); within a cell order is irrelevant
    key = (core * Q + q) * NBLK + b
    order = np.argsort(key, kind="stable")
    src_s, dloc_s, attr_s, key_s = src[order], dloc[order], attr[order], key[order]

    # per-(core, q, b) counts -> uniform tile capacity per cell
    counts = np.bincount(key_s, minlength=NCORES * Q * NBLK)
    tqb = int((counts.max() + P - 1) // P)
    tqb = max(tqb, 1)

    tiles_per_q = NBLK * tqb
    T = Q * tiles_per_q                  # tiles per core
    SLOTS = T * P                        # edge slots per core
    cell_cap = tqb * P

    # slot id within a core: cell (q, b) occupies [ (q*NBLK+b)*cell_cap, ... )
    cell_ix = np.zeros(NCORES * Q * NBLK + 1, dtype=np.int64)
    cell_ix[1:] = np.cumsum(counts)
    # position of each (sorted) edge within its cell
    pos_in_cell = np.arange(E, dtype=np.int64) - cell_ix[key_s]
    cell_of_edge = key_s
    core_of_edge = cell_of_edge // (Q * NBLK)
    celllocal = cell_of_edge % (Q * NBLK)
    slot = celllocal * cell_cap + pos_in_cell   # slot within the core

    idx16 = np.zeros((NCORES, SLOTS), dtype=np.int16)
    dstrel = np.full((NCORES, SLOTS), -1.0, dtype=BF16)
    attr1h = np.zeros((NCORES, 4, SLOTS), dtype=BF16)

    c_arr = core_of_edge
    idx16[c_arr, slot] = (src_s - q[order] * QS).astype(np.int16)
    dstrel[c_arr, slot] = (dloc_s & 127).astype(np.float32).astype(BF16)
    attr1h[c_arr, attr_s, slot] = BF16(1.0)

    # wrap idx16 per gather call: slot i (within call) -> [i % 16, i // 16]
    calls_q = 14 if (tiles_per_q % 14 == 0) else 7
    while tiles_per_q % calls_q != 0:
        calls_q -= 1
    call_tiles = tiles_per_q // calls_q
    call_slots = call_tiles * P
    if call_slots > 8192:
        # split calls further if the per-call index count gets too large
        for cq in range(calls_q + 1, tiles_per_q + 1):
            if tiles_per_q % cq == 0 and (tiles_per_q // cq) * P <= 8192:
                calls_q = cq
                call_tiles = tiles_per_q // cq
                call_slots = call_tiles * P
                break
    ncalls = Q * calls_q

    idx_w = idx16.reshape(NCORES, ncalls, call_slots // 16, 16)
    idx_w = np.ascontiguousarray(np.swapaxes(idx_w, 2, 3))  # [NC, ncalls, 16, cs/16]
    idx_wrapped = idx_w.reshape(NCORES, ncalls * 16, call_slots // 16)
    # final DRAM layout: [128, SLOTS/16] where call c occupies columns
    # [c*cs/16, (c+1)*cs/16) and its 16 rows are replicated 8x down partitions
    idx_dram = np.zeros((NCORES, P, SLOTS // 16), dtype=np.int16)
    for c in range(ncalls):
        blkc = idx_wrapped[:, c * 16:(c + 1) * 16, :]           # [NC, 16, cs/16]
        cols = slice(c * (call_slots // 16), (c + 1) * (call_slots // 16))
        idx_dram[:, :, cols] = np.tile(blkc, (1, 8, 1))

    dstrel_mat = np.ascontiguousarray(
        dstrel.reshape(NCORES, T, P).swapaxes(1, 2))       # [NC, 128, T]

    meta = dict(tqb=tqb, T=T, SLOTS=SLOTS, calls_q=calls_q,
                call_tiles=call_tiles, call_slots=call_slots)
    return idx_dram, dstrel_mat, attr1h, meta


def _build_program(meta, emb_lo_split=True):
    import concourse.bass as bass
    import concourse.bacc as bacc
    import concourse.mybir as mybir
    import concourse.tile as tile

    dt = mybir.dt
    Alu = mybir.AluOpType
    Act = mybir.ActivationFunctionType

    tqb = meta["tqb"]
    T = meta["T"]
    SLOTS = meta["SLOTS"]
    calls_q = meta["calls_q"]
    call_tiles = meta["call_tiles"]
    call_slots = meta["call_slots"]
    CHUNK = 7 if call_tiles % 7 == 0 else 1
    while call_tiles % CHUNK != 0:
        CHUNK -= 1
    chunks_per_call = call_tiles // CHUNK

    nc = bacc.Bacc("TRN2", target_bir_lowering=False, debug=False,
                   num_devices=NCORES)
    _aggps = {}

    f32, bf16, i16 = dt.float32, dt.bfloat16, dt.int16

    xg = nc.dram_tensor("xg", [XROWS, IN_DIM], f32, kind="ExternalInput")
    xloc = nc.dram_tensor("xloc", [NLOC, IN_DIM], f32, kind="ExternalInput")
    idx16 = nc.dram_tensor("idx16", [P, SLOTS // 16], i16, kind="ExternalInput")
    dstrel = nc.dram_tensor("dstrel", [P, T], bf16, kind="ExternalInput")
    attr1h = nc.dram_tensor("attr1h", [4, SLOTS], bf16, kind="ExternalInput")
    iota_d = nc.dram_tensor("iota", [P, P], bf16, kind="ExternalInput")
    ident_d = nc.dram_tensor("ident", [P, P], f32, kind="ExternalInput")
    emb_d = nc.dram_tensor("emb", [4, IN_DIM], f32, kind="ExternalInput")
    w1_d = nc.dram_tensor("w1", [IN_DIM, IN_DIM], f32, kind="ExternalInput")
    w2_d = nc.dram_tensor("w2", [IN_DIM, IN_DIM], f32, kind="ExternalInput")
    gam_d = nc.dram_tensor("gam", [IN_DIM, 1], f32, kind="ExternalInput")
    bet_d = nc.dram_tensor("bet", [IN_DIM, 1], f32, kind="ExternalInput")
    b2_d = nc.dram_tensor("b2", [IN_DIM, 1], f32, kind="ExternalInput")
    eps_d = nc.dram_tensor("eps", [P, 1], f32, kind="ExternalInput")
    out_d = nc.dram_tensor("outT", [IN_DIM, NLOC], f32, kind="ExternalOutput")

    cc_in = nc.dram_tensor("cc_in", [IN_DIM, 2], f32)
    cc_out = nc.dram_tensor("cc_out", [IN_DIM, 2], f32, addr_space="Shared")

    with tile.TileContext(nc) as tc:
        with (
            tc.tile_pool(name="const", bufs=1) as cpool,
            tc.tile_pool(name="big", bufs=1) as bigpool,
            tc.tile_pool(name="gin", bufs=2) as gpool,
            tc.tile_pool(name="meta", bufs=2) as mpool,
            tc.tile_pool(name="work", bufs=3) as wpool,
            tc.tile_pool(name="psum", bufs=2, space="PSUM") as pp,
            tc.tile_pool(name="psagg", bufs=4, space="PSUM") as ppagg,
        ):
            # ---- constants / params ----
            iota_t = cpool.tile([P, P], bf16)
            nc.sync.dma_start(out=iota_t[:], in_=iota_d[:])
            ident_t = cpool.tile([P, P], f32)
            nc.sync.dma_start(out=ident_t[:], in_=ident_d[:])
            dstrel_t = cpool.tile([P, T], bf16)
            nc.sync.dma_start(out=dstrel_t[:], in_=dstrel[:])
            emb_t = cpool.tile([4, IN_DIM], f32)
            nc.sync.dma_start(out=emb_t[:], in_=emb_d[:])
            w1_t = cpool.tile([IN_DIM, IN_DIM], f32)
            nc.sync.dma_start(out=w1_t[:], in_=w1_d[:])
            w2_t = cpool.tile([IN_DIM, IN_DIM], f32)
            nc.sync.dma_start(out=w2_t[:], in_=w2_d[:])
            gam_t = cpool.tile([IN_DIM, 1], f32)
            nc.sync.dma_start(out=gam_t[:], in_=gam_d[:])
            bet_t = cpool.tile([IN_DIM, 1], f32)
            nc.sync.dma_start(out=bet_t[:], in_=bet_d[:])
            b2_t = cpool.tile([IN_DIM, 1], f32)
            nc.sync.dma_start(out=b2_t[:], in_=b2_d[:])
            eps_t = cpool.tile([P, 1], f32)
            nc.sync.dma_start(out=eps_t[:], in_=eps_d[:])

            # emb hi/lo bf16 split (exact to ~2^-17)
            emb_hi = cpool.tile([4, IN_DIM], bf16)
            nc.vector.tensor_copy(out=emb_hi[:], in_=emb_t[:])
            emb_lo = cpool.tile([4, IN_DIM], bf16)
            if emb_lo_split:
                emb_hi_f = cpool.tile([4, IN_DIM], f32)
                nc.vector.tensor_copy(out=emb_hi_f[:], in_=emb_hi[:])
                emb_lo_f = cpool.tile([4, IN_DIM], f32)
                nc.vector.tensor_tensor(out=emb_lo_f[:], in0=emb_t[:],
                                        in1=emb_hi_f[:], op=Alu.subtract)
                nc.vector.tensor_copy(out=emb_lo[:], in_=emb_lo_f[:])

            # Ieps = (1 + eps) * I  (f32, exact for any eps)
            eps1_t = cpool.tile([P, 1], f32)
            nc.scalar.activation(out=eps1_t[:], in_=eps_t[:], func=Act.Identity,
                                 bias=1.0, scale=1.0)
            ieps_t = cpool.tile([P, P], f32)
            nc.vector.tensor_scalar(ieps_t[:], ident_t[:], eps1_t[:, :1], None,
                                    Alu.mult)

            # aggregated h^T accumulator [64, NLOC]
            agg_sb = bigpool.tile([IN_DIM, NLOC], f32)
            nc.vector.memset(agg_sb[:], 0.0)

            # ---- edge phase ----
            for qi in range(Q):
                for ci in range(calls_q):
                    call = qi * calls_q + ci
                    t0 = call * call_tiles          # first tile of this call
                    s0 = t0 * P                     # first slot
                    idx_sb = mpool.tile([P, call_slots // 16], i16, tag="idx")
                    nc.sync.dma_start(
                        out=idx_sb[:],
                        in_=idx16[:, call * (call_slots // 16):
                                  (call + 1) * (call_slots // 16)])
                    gbuf = gpool.tile([P, call_tiles * IN_DIM], f32, tag="g")
                    nc.gpsimd.dma_gather(
                        out_ap=gbuf[:].rearrange("p (k f) -> p k f", f=IN_DIM),
                        in_ap=xg[qi * QS:qi * QS + QW, :],
                        idxs_ap=idx_sb[:],
                        num_idxs=call_slots,
                        num_idxs_reg=call_slots,
                        elem_size=IN_DIM,
                        single_packet=False,
                    )
                    for ch in range(chunks_per_call):
                        # tiles [t0 + ch*CHUNK, t0 + (ch+1)*CHUNK)
                        tb = t0 + ch * CHUNK
                        at_sb = mpool.tile([4, CHUNK * P], bf16, tag="attr")
                        nc.sync.dma_start(
                            out=at_sb[:],
                            in_=attr1h[:, tb * P:(tb + CHUNK) * P])
                        ps_pre = pp.tile([P, CHUNK * IN_DIM], f32, space="PSUM",
                                         tag="pre")
                        for j in range(CHUNK):
                            acol = j * P
                            lhs = at_sb[:, acol:acol + P]
                            nc.tensor.matmul(
                                out=ps_pre[:, j * IN_DIM:(j + 1) * IN_DIM],
                                lhsT=lhs, rhs=emb_hi[:],
                                start=True, stop=not emb_lo_split)
                            if emb_lo_split:
                                nc.tensor.matmul(
                                    out=ps_pre[:, j * IN_DIM:(j + 1) * IN_DIM],
                                    lhsT=lhs, rhs=emb_lo[:],
                                    start=False, stop=True)
                        # msg = relu(gx + emb_one_hot @ emb)
                        gsl = gbuf[:, ch * CHUNK * IN_DIM:
                                   (ch + 1) * CHUNK * IN_DIM]
                        madd = wpool.tile([P, CHUNK * IN_DIM], f32, tag="madd")
                        nc.vector.tensor_tensor(out=madd[:], in0=gsl,
                                                in1=ps_pre[:], op=Alu.add)
                        msg = wpool.tile([P, CHUNK * IN_DIM], bf16, tag="msg")
                        nc.scalar.activation(out=msg[:], in_=madd[:],
                                             func=Act.Relu)
                        # S[e, j] = (dstrel_e == j), bf16 0/1
                        s_t = wpool.tile([P, CHUNK * P], bf16, tag="S")
                        io_b = iota_t[:].rearrange("p (o f) -> p o f", o=1).to_broadcast(
                            [P, CHUNK, P])
                        dr_b = dstrel_t[:, tb:tb + CHUNK].rearrange(
                            "p (k o) -> p k o", o=1).to_broadcast([P, CHUNK, P])
                        nc.vector.tensor_tensor(
                            out=s_t[:].rearrange("p (k f) -> p k f", f=P),
                            in0=io_b, in1=dr_b, op=Alu.is_equal)
                        for j in range(CHUNK):
                            t = tb + j
                            tq = t % tqb            # tile index within cell
                            cell = t // tqb
                            blk = cell % NBLK
                            if tq == 0:
                                cur = ppagg.tile([IN_DIM, P], f32, space="PSUM",
                                                 tag="agg")
                                _aggps[blk] = cur
                            cur = _aggps[blk]
                            last = (tq == tqb - 1)
                            nc.tensor.matmul(
                                out=cur[:],
                                lhsT=msg[:, j * IN_DIM:(j + 1) * IN_DIM],
                                rhs=s_t[:, j * P:(j + 1) * P],
                                start=(tq == 0),
                                stop=last and qi != Q - 1)
                            if last and qi == Q - 1:
                                # fold in (1+eps)*x for this block
                                xb = wpool.tile([P, IN_DIM], f32, tag="xb")
                                nc.sync.dma_start(
                                    out=xb[:],
                                    in_=xloc[blk * P:(blk + 1) * P, :])
                                nc.tensor.matmul(out=cur[:], lhsT=xb[:],
                                                 rhs=ieps_t[:],
                                                 start=False, stop=True)
                            if last:
                                eng = nc.vector
                                asl = agg_sb[:, blk * P:(blk + 1) * P]
                                eng.tensor_tensor(out=asl, in0=asl, in1=cur[:],
                                                  op=Alu.add)

            # ---- MLP tail (feature-major) ----
            MT = 512
            nmt = (NLOC + MT - 1) // MT
            h1_sb = bigpool.tile([IN_DIM, NLOC], f32)
            sum_parts = cpool.tile([IN_DIM, nmt], f32)
            sq_parts = cpool.tile([IN_DIM, nmt], f32)
            sq_scratch = wpool.tile([IN_DIM, MT], f32, tag="sqs")
            for m in range(nmt):
                lo = m * MT
                w = min(MT, NLOC - lo)
                ps1 = pp.tile([IN_DIM, MT], f32, space="PSUM", tag="mlp")
                nc.tensor.matmul(out=ps1[:, :w], lhsT=w1_t[:],
                                 rhs=agg_sb[:, lo:lo + w], start=True, stop=True)
                nc.scalar.activation(out=h1_sb[:, lo:lo + w], in_=ps1[:, :w],
                                     func=Act.Identity, bias=0.0, scale=1.0,
                                     accum_out=sum_parts[:, m:m + 1])
                nc.scalar.activation(out=sq_scratch[:, :w], in_=h1_sb[:, lo:lo + w],
                                     func=Act.Square,
                                     accum_out=sq_parts[:, m:m + 1])
            sums = cpool.tile([IN_DIM, 1], f32)
            nc.vector.tensor_reduce(out=sums[:], in_=sum_parts[:],
                                    axis=mybir.AxisListType.X, op=Alu.add)
            sqs = cpool.tile([IN_DIM, 1], f32)
            nc.vector.tensor_reduce(out=sqs[:], in_=sq_parts[:],
                                    axis=mybir.AxisListType.X, op=Alu.add)
            stats = cpool.tile([IN_DIM, 2], f32)
            nc.vector.tensor_copy(out=stats[:, 0:1], in_=sums[:])
            nc.vector.tensor_copy(out=stats[:, 1:2], in_=sqs[:])
            nc.gpsimd.dma_start(out=cc_in[:], in_=stats[:])
            nc.gpsimd.collective_compute(
                "AllReduce", Alu.add,
                replica_groups=[list(range(NCORES))],
                ins=[cc_in[:]], outs=[cc_out[:]])
            astats = cpool.tile([IN_DIM, 2], f32)
            nc.gpsimd.dma_start(out=astats[:], in_=cc_out[:])

            inv_n = 1.0 / float(N_NODES)
            mu = cpool.tile([IN_DIM, 1], f32)
            nc.vector.tensor_scalar(mu[:], astats[:, 0:1], inv_n, None, Alu.mult)
            ex2 = cpool.tile([IN_DIM, 1], f32)
            nc.vector.tensor_scalar(ex2[:], astats[:, 1:2], inv_n, None, Alu.mult)
            mu2 = cpool.tile([IN_DIM, 1], f32)
            nc.vector.tensor_tensor(out=mu2[:], in0=mu[:], in1=mu[:], op=Alu.mult)
            vare = cpool.tile([IN_DIM, 1], f32)
            nc.vector.tensor_tensor(out=vare[:], in0=ex2[:], in1=mu2[:],
                                    op=Alu.subtract)
            nc.vector.tensor_scalar(vare[:], vare[:], BN_EPS, None, Alu.add)
            rvar = cpool.tile([IN_DIM, 1], f32)
            nc.vector.reciprocal(out=rvar[:], in_=vare[:])
            rstd = cpool.tile([IN_DIM, 1], f32)
            nc.scalar.activation(out=rstd[:], in_=rvar[:], func=Act.Sqrt)
            rg = cpool.tile([IN_DIM, 1], f32)
            nc.vector.tensor_tensor(out=rg[:], in0=rstd[:], in1=gam_t[:],
                                    op=Alu.mult)
            murg = cpool.tile([IN_DIM, 1], f32)
            nc.vector.tensor_tensor(out=murg[:], in0=mu[:], in1=rg[:],
                                    op=Alu.mult)
            bmr = cpool.tile([IN_DIM, 1], f32)
            nc.vector.tensor_tensor(out=bmr[:], in0=bet_t[:], in1=murg[:],
                                    op=Alu.subtract)

            for m in range(nmt):
                lo = m * MT
                w = min(MT, NLOC - lo)
                hr = wpool.tile([IN_DIM, MT], f32, tag="hr")
                nc.scalar.activation(out=hr[:, :w], in_=h1_sb[:, lo:lo + w],
                                     func=Act.Relu, bias=bmr[:, :1],
                                     scale=rg[:, :1])
                ps2 = pp.tile([IN_DIM, MT], f32, space="PSUM", tag="mlp")
                nc.tensor.matmul(out=ps2[:, :w], lhsT=w2_t[:], rhs=hr[:, :w],
                                 start=True, stop=True)
                nc.scalar.activation(out=agg_sb[:, lo:lo + w], in_=ps2[:, :w],
                                     func=Act.Identity, bias=b2_t[:, :1],
                                     scale=1.0)
            nc.sync.dma_start(out=out_d[:], in_=agg_sb[:])

    nc.compile()
    return nc


def _install_ntff_hook():
    """Best-effort NTFF profiling hook (axon terminal). Trace-mode only."""
    import types
    try:
        import antenv
        if not hasattr(antenv, "axon_hooks"):
            m = types.ModuleType("antenv.axon_hooks")
            m._hook = None
            m.set_axon_ntff_profile_hook = lambda h: setattr(m, "_hook", h)
            m.get_axon_ntff_profile_hook = lambda: m._hook
            sys.modules["antenv.axon_hooks"] = m
            antenv.axon_hooks = m
        from antenv import axon_hooks
        if axon_hooks.get_axon_ntff_profile_hook() is None:
            from trn_agent_boot.trn_boot import _ntff_profile_via_ctypes
            h = _ntff_profile_via_ctypes("/opt/axon/libaxon_pjrt.so")
            if h is not None:
                axon_hooks.set_axon_ntff_profile_hook(h)
    except Exception as e:
        print("ntff hook install failed:", e)


def kernel(**inputs):
    x = np.ascontiguousarray(np.asarray(inputs["x"], dtype=np.float32))
    edge_index = np.asarray(inputs["edge_index"])
    edge_attr = np.asarray(inputs["edge_attr"])
    emb = np.ascontiguousarray(np.asarray(inputs["edge_emb_table"], np.float32))
    eps = float(np.asarray(inputs["eps"], np.float32))
    W1 = np.ascontiguousarray(np.asarray(inputs["W1"], np.float32))
    b1 = np.asarray(inputs["b1"], np.float32)  # cancels in BatchNorm; unused
    gamma = np.asarray(inputs["gamma"], np.float32)
    beta = np.asarray(inputs["beta"], np.float32)
    W2 = np.ascontiguousarray(np.asarray(inputs["W2"], np.float32))
    b2 = np.asarray(inputs["b2"], np.float32)

    idx_dram, dstrel_mat, attr1h, meta = _host_prepare(x, edge_index, edge_attr)

    key = (meta["tqb"], meta["T"], meta["calls_q"])
    if key not in _PROGRAM_CACHE:
        _PROGRAM_CACHE[key] = _build_program(meta)
    nc = _PROGRAM_CACHE[key]

    xg = np.zeros((XROWS, IN_DIM), np.float32)
    xg[:N_NODES] = x
    iota = np.tile(np.arange(P, dtype=np.float32), (P, 1)).astype(BF16)
    ident = np.eye(P, dtype=np.float32)
    eps_col = np.full((P, 1), eps, np.float32)

    in_maps = []
    for c in range(NCORES):
        xloc = np.zeros((NLOC, IN_DIM), np.float32)
        xloc[:NPC] = x[c * NPC:(c + 1) * NPC]
        in_maps.append({
            "xg": xg,
            "xloc": xloc,
            "idx16": idx_dram[c],
            "dstrel": dstrel_mat[c],
            "attr1h": attr1h[c],
            "iota": iota,
            "ident": ident,
            "emb": emb,
            "w1": W1,
            "w2": W2,
            "gam": np.ascontiguousarray(gamma.reshape(IN_DIM, 1)),
            "bet": np.ascontiguousarray(beta.reshape(IN_DIM, 1)),
            "b2": np.ascontiguousarray(b2.reshape(IN_DIM, 1)),
            "eps": eps_col,
        })

    from concourse.bass_utils import run_bass_kernel_spmd
    trace = os.environ.get("BASS_GNN_TRACE", "0") == "1"
    if trace:
        _install_ntff_hook()

    res = run_bass_kernel_spmd(nc, in_maps, core_ids=list(range(NCORES)),
                               trace=trace)
    kernel.last_exec_time_ns = res.exec_time_ns

    out = np.empty((N_NODES, IN_DIM), np.float32)
    for c in range(NCORES):
        out[c * NPC:(c + 1) * NPC] = res.results[c]["outT"][:, :NPC].T
    return out
